# revision 9
# baseline (speedup 1.0000x reference)
"""ForgetMult recurrence h_t = f_t*x_t + (1-f_t)*h_{t-1} on 8 TRN2 NeuronCores.

Strategy
--------
Shard batch (dim 1) across the 8 cores: each core owns [T=512, B=8, H=1024]
= 8192 independent recurrence lanes of length 512.

The kernel is HBM-bandwidth-bound (3 tensor-sized transfers/core at 358 GB/s),
so the wire format is fp16: the host casts f and x to fp16 and packs them as
[p=lane%128, block=lane//128, t] so each lane's full time series lies along
the free dim of one SBUF partition — the layout tensor_tensor_scan needs —
with zero on-device transposes. The device streams 8 chunks of 8 lane-blocks:

  DMA in   f,x chunk [128, 8*512] fp16 (8KB/partition contiguous lines)
  DVE      b = f*x            (fp16 2x mode)
  ACT      a = 1 - f          (activation copy, scale=-1 bias=1)
  DVE      h = scan(a, b, h0) per 512-t block (fp32 internal state)
  DMA out  h chunk fp16

The host unpacks the fp16 output back to [T, B, H] f32. Accuracy: the
recurrence is a convex combination (contraction), so fp16 rounding stays
~1e-3 total, far under the 2e-2 gate. HBM traffic/core: 25.2 MB -> ~70us
floor vs 142us for the f32 format.
"""

import sys

if "/opt/trn_rl_repo" not in sys.path:
    sys.path.insert(0, "/opt/trn_rl_repo")

from contextlib import ExitStack

import numpy as np

import concourse.tile as tile
from concourse import bacc, mybir
from concourse.bass_utils import run_bass_kernel_spmd

T, B, H = 512, 64, 1024
NCORES = 8
BS = B // NCORES          # batch rows per core
L = BS * H                # lanes per core
P = 128                   # SBUF partitions
NBLK = L // P             # 64 lane blocks of 128 lanes
NB = 8                    # lane blocks per chunk
NCH = NBLK // NB          # chunks per core
F16 = mybir.dt.float16
F32 = mybir.dt.float32
MULT = mybir.AluOpType.mult
ADD = mybir.AluOpType.add
COPY = mybir.ActivationFunctionType.Copy

_PROGRAM = None


def build_program(repeat=1, out_eng="scalar", kpool=4):
    # out_eng: engine whose HWDGE queue carries the store DMAs (loads stay on
    # the SP queue; splitting roughly balances the two queues).
    # kpool: how many of the 8 chunks run their f*x mult on GpSimd/Pool
    # instead of DVE. The scans are DVE-only (neuronxcc rejects
    # tensor_tensor_scan on Pool), so DVE holds 34us of scans; shifting mult
    # chunks to the otherwise-idle Pool (~8.1us/chunk at 0.42 sw efficiency
    # vs 2.1us on DVE) trades cheap Pool time for scarce DVE time.
    nc = bacc.Bacc(
        "TRN2",
        debug=False,
        enable_asserts=False,
        target_bir_lowering=False,
        num_devices=NCORES,
    )
    f_d = nc.dram_tensor("f", [P, NBLK, T], F16, kind="ExternalInput").ap()
    x_d = nc.dram_tensor("x", [P, NBLK, T], F16, kind="ExternalInput").ap()
    h0_d = nc.dram_tensor("hidden_init", [P, NBLK], F32, kind="ExternalInput").ap()
    o_d = nc.dram_tensor("out", [P, NBLK, T], F16, kind="ExternalOutput").ap()

    with tile.TileContext(nc) as tc, ExitStack() as ctx:
        const = ctx.enter_context(tc.tile_pool(name="const", bufs=1))
        fpool = ctx.enter_context(tc.tile_pool(name="fpool", bufs=3))
        xpool = ctx.enter_context(tc.tile_pool(name="xpool", bufs=3))
        bpool = ctx.enter_context(tc.tile_pool(name="bpool", bufs=2))
        apool = ctx.enter_context(tc.tile_pool(name="apool", bufs=2))
        hpool = ctx.enter_context(tc.tile_pool(name="hpool", bufs=2))

        h0t = const.tile([P, NBLK], F32)
        nc.sync.dma_start(h0t[:], h0_d[:, :])

        # kpool chunks spread evenly across the 8 run their mult on Pool
        pool_chunks = {i * NCH // kpool for i in range(kpool)} if kpool else set()

        for rep in range(repeat):
            for ch in range(NCH):
                b0 = ch * NB
                ft = fpool.tile([P, NB * T], F16, tag="f", name=f"f_{rep}_{ch}")
                xt = xpool.tile([P, NB * T], F16, tag="x", name=f"x_{rep}_{ch}")
                nc.sync.dma_start(
                    ft.rearrange("p (b t) -> p b t", b=NB), f_d[:, b0 : b0 + NB, :]
                )
                nc.sync.dma_start(
                    xt.rearrange("p (b t) -> p b t", b=NB), x_d[:, b0 : b0 + NB, :]
                )
                bt = bpool.tile([P, NB * T], F16, tag="b", name=f"b_{rep}_{ch}")
                on_pool = kpool > 0 and ch in pool_chunks
                (nc.gpsimd if on_pool else nc.vector).tensor_tensor(
                    bt[:], ft[:], xt[:], MULT
                )
                at = apool.tile([P, NB * T], F16, tag="a", name=f"a_{rep}_{ch}")
                nc.scalar.activation(at[:], ft[:], COPY, bias=1.0, scale=-1.0)
                ht = hpool.tile([P, NB * T], F16, tag="h", name=f"h_{rep}_{ch}")
                for j in range(NB):
                    sl = slice(j * T, (j + 1) * T)
                    nc.vector.tensor_tensor_scan(
                        ht[:, sl],
                        at[:, sl],
                        bt[:, sl],
                        h0t[:, b0 + j : b0 + j + 1],
                        MULT,
                        ADD,
                    )
                getattr(nc, out_eng).dma_start(
                    o_d[:, b0 : b0 + NB, :], ht.rearrange("p (b t) -> p b t", b=NB)
                )

    nc.compile()
    return nc


def get_program():
    global _PROGRAM
    if _PROGRAM is None:
        _PROGRAM = build_program()
    return _PROGRAM


def _pack(a2d):
    # [T, L] f32 -> [P, NBLK, T] fp16, lane l = blk*P + p
    return np.ascontiguousarray(
        a2d.reshape(T, NBLK, P).transpose(2, 1, 0), dtype=np.float16
    )


def make_in_maps(f, x, h0):
    maps = []
    for c in range(NCORES):
        sl = slice(c * BS, (c + 1) * BS)
        maps.append(
            {
                "f": _pack(f[:, sl, :].reshape(T, L)),
                "x": _pack(x[:, sl, :].reshape(T, L)),
                "hidden_init": np.ascontiguousarray(
                    h0[sl, :].reshape(NBLK, P).T, dtype=np.float32
                ),
            }
        )
    return maps


def unpack_out(o_packed):
    # [P, NBLK, T] fp16 -> [T, BS, H] f32
    return o_packed.transpose(2, 1, 0).reshape(T, BS, H).astype(np.float32)


def kernel(**inputs):
    f = np.asarray(inputs["f"], dtype=np.float32)
    x = np.asarray(inputs["x"], dtype=np.float32)
    h0 = np.asarray(inputs["hidden_init"], dtype=np.float32)
    assert f.shape == (T, B, H) and x.shape == (T, B, H) and h0.shape == (B, H)

    nc = get_program()
    res = run_bass_kernel_spmd(nc, make_in_maps(f, x, h0), list(range(NCORES)))
    return np.concatenate(
        [unpack_out(res.results[c]["out"]) for c in range(NCORES)], axis=1
    )


# revision 11
# speedup vs baseline: 2.3324x; 2.3324x over previous
"""ForgetMult recurrence h_t = f_t*x_t + (1-f_t)*h_{t-1} on 8 TRN2 NeuronCores.

Strategy
--------
Shard batch (dim 1) across the 8 cores: each core owns [T=512, B=8, H=1024]
= 8192 independent recurrence lanes of length 512.

The kernel is HBM-bandwidth-bound (3 tensor-sized transfers/core at 358 GB/s),
so the wire format is fp16: the host casts f and x to fp16 and packs them as
[p=lane%128, block=lane//128, t] so each lane's full time series lies along
the free dim of one SBUF partition — the layout tensor_tensor_scan needs —
with zero on-device transposes. The device streams 8 chunks of 8 lane-blocks:

  DMA in   f,x chunk [128, 8*512] fp16 (8KB/partition contiguous lines)
  DVE      b = f*x            (fp16 2x mode)
  ACT      a = 1 - f          (activation copy, scale=-1 bias=1)
  DVE      h = scan(a, b, h0) per 512-t block (fp32 internal state)
  DMA out  h chunk fp16

The host unpacks the fp16 output back to [T, B, H] f32. Accuracy: the
recurrence is a convex combination (contraction), so fp16 rounding stays
~1e-3 total, far under the 2e-2 gate. HBM traffic/core: 25.2 MB -> ~70us
floor vs 142us for the f32 format.
"""

import sys

if "/opt/trn_rl_repo" not in sys.path:
    sys.path.insert(0, "/opt/trn_rl_repo")

from contextlib import ExitStack

import numpy as np

import concourse.tile as tile
from concourse import bacc, mybir
from concourse.bass_utils import run_bass_kernel_spmd

T, B, H = 512, 64, 1024
NCORES = 8
BS = B // NCORES          # batch rows per core
L = BS * H                # lanes per core
P = 128                   # SBUF partitions
NBLK = L // P             # 64 lane blocks of 128 lanes
NB = 8                    # lane blocks per chunk
NCH = NBLK // NB          # chunks per core
F16 = mybir.dt.float16
F32 = mybir.dt.float32
MULT = mybir.AluOpType.mult
ADD = mybir.AluOpType.add
COPY = mybir.ActivationFunctionType.Copy

_PROGRAM = None


def build_program(repeat=1, out_eng="scalar", kpool=0, bufs=(4, 4, 3, 3, 3)):
    # out_eng: engine whose HWDGE queue carries the store DMAs (loads stay on
    # the SP queue; splitting roughly balances the two queues).
    # kpool: chunks whose f*x mult runs on GpSimd/Pool instead of DVE.
    # Measured 0.15 sw efficiency (~22us/chunk vs 2.1 on DVE) — Pool-bound at
    # kpool=4 (96us), so default 0. Scans are DVE-only (neuronxcc rejects
    # tensor_tensor_scan on Pool), which pins DVE at ~56us busy; the DMA
    # floor is 48.8us, so the kernel is DVE-bound and deep buffering only
    # needs to hide the pipeline fill/drain.
    nc = bacc.Bacc(
        "TRN2",
        debug=False,
        enable_asserts=False,
        target_bir_lowering=False,
        num_devices=NCORES,
    )
    f_d = nc.dram_tensor("f", [P, NBLK, T], F16, kind="ExternalInput").ap()
    x_d = nc.dram_tensor("x", [P, NBLK, T], F16, kind="ExternalInput").ap()
    h0_d = nc.dram_tensor("hidden_init", [P, NBLK], F32, kind="ExternalInput").ap()
    o_d = nc.dram_tensor("out", [P, NBLK, T], F16, kind="ExternalOutput").ap()

    with tile.TileContext(nc) as tc, ExitStack() as ctx:
        const = ctx.enter_context(tc.tile_pool(name="const", bufs=1))
        fpool = ctx.enter_context(tc.tile_pool(name="fpool", bufs=bufs[0]))
        xpool = ctx.enter_context(tc.tile_pool(name="xpool", bufs=bufs[1]))
        bpool = ctx.enter_context(tc.tile_pool(name="bpool", bufs=bufs[2]))
        apool = ctx.enter_context(tc.tile_pool(name="apool", bufs=bufs[3]))
        hpool = ctx.enter_context(tc.tile_pool(name="hpool", bufs=bufs[4]))

        h0t = const.tile([P, NBLK], F32)
        nc.sync.dma_start(h0t[:], h0_d[:, :])

        # kpool chunks spread evenly across the 8 run their mult on Pool
        pool_chunks = {i * NCH // kpool for i in range(kpool)} if kpool else set()

        for rep in range(repeat):
            for ch in range(NCH):
                b0 = ch * NB
                ft = fpool.tile([P, NB * T], F16, tag="f", name=f"f_{rep}_{ch}")
                xt = xpool.tile([P, NB * T], F16, tag="x", name=f"x_{rep}_{ch}")
                nc.sync.dma_start(
                    ft.rearrange("p (b t) -> p b t", b=NB), f_d[:, b0 : b0 + NB, :]
                )
                nc.sync.dma_start(
                    xt.rearrange("p (b t) -> p b t", b=NB), x_d[:, b0 : b0 + NB, :]
                )
                bt = bpool.tile([P, NB * T], F16, tag="b", name=f"b_{rep}_{ch}")
                on_pool = kpool > 0 and ch in pool_chunks
                (nc.gpsimd if on_pool else nc.vector).tensor_tensor(
                    bt[:], ft[:], xt[:], MULT
                )
                at = apool.tile([P, NB * T], F16, tag="a", name=f"a_{rep}_{ch}")
                nc.scalar.activation(at[:], ft[:], COPY, bias=1.0, scale=-1.0)
                ht = hpool.tile([P, NB * T], F16, tag="h", name=f"h_{rep}_{ch}")
                for j in range(NB):
                    sl = slice(j * T, (j + 1) * T)
                    nc.vector.tensor_tensor_scan(
                        ht[:, sl],
                        at[:, sl],
                        bt[:, sl],
                        h0t[:, b0 + j : b0 + j + 1],
                        MULT,
                        ADD,
                    )
                getattr(nc, out_eng).dma_start(
                    o_d[:, b0 : b0 + NB, :], ht.rearrange("p (b t) -> p b t", b=NB)
                )

    nc.compile()
    return nc


def get_program():
    global _PROGRAM
    if _PROGRAM is None:
        _PROGRAM = build_program()
    return _PROGRAM


def _pack(a2d):
    # [T, L] f32 -> [P, NBLK, T] fp16, lane l = blk*P + p
    return np.ascontiguousarray(
        a2d.reshape(T, NBLK, P).transpose(2, 1, 0), dtype=np.float16
    )


def make_in_maps(f, x, h0):
    maps = []
    for c in range(NCORES):
        sl = slice(c * BS, (c + 1) * BS)
        maps.append(
            {
                "f": _pack(f[:, sl, :].reshape(T, L)),
                "x": _pack(x[:, sl, :].reshape(T, L)),
                "hidden_init": np.ascontiguousarray(
                    h0[sl, :].reshape(NBLK, P).T, dtype=np.float32
                ),
            }
        )
    return maps


def unpack_out(o_packed):
    # [P, NBLK, T] fp16 -> [T, BS, H] f32
    return o_packed.transpose(2, 1, 0).reshape(T, BS, H).astype(np.float32)


def kernel(**inputs):
    f = np.asarray(inputs["f"], dtype=np.float32)
    x = np.asarray(inputs["x"], dtype=np.float32)
    h0 = np.asarray(inputs["hidden_init"], dtype=np.float32)
    assert f.shape == (T, B, H) and x.shape == (T, B, H) and h0.shape == (B, H)

    nc = get_program()
    res = run_bass_kernel_spmd(nc, make_in_maps(f, x, h0), list(range(NCORES)))
    return np.concatenate(
        [unpack_out(res.results[c]["out"]) for c in range(NCORES)], axis=1
    )
